# revision 2
# baseline (speedup 1.0000x reference)
"""Tensor-parallel GPT-J-style attention block on 8 TRN2 NeuronCores. v4.

v3 schedule fixes (QKV weight prefetch, gpsimd-broadcast normalize, early
PSUM eviction, hl-split output projection) plus:
  - the attention output, AllToAll buffers, and w_out/per-core attn tiles
    are bf16: halves the only collective's volume and the w_out read
    (adds ~1e-3 relative error; PSUM accumulation stays fp32).
  - output projection issues each hl's first w blocks ahead of the
    collective-gated attn-tile loads so DMA queues never head-of-line
    block on the second AllToAll.
"""
import math
import sys

import numpy as np

try:
    import concourse.bass  # noqa: F401
except ImportError:
    sys.path.insert(0, "/opt/trn_rl_repo")

import concourse.mybir as mybir
import concourse.tile as tile
from concourse import bacc
from concourse.bass_utils import run_bass_kernel_spmd
from concourse.masks import make_identity, make_upper_triangular

dt = mybir.dt

N_CORES = 8
B = 4
NH = 16
D = 256
HID = NH * D  # 4096
ROT = D // 2  # 128
RH = ROT // 2  # 64
HPC = NH // N_CORES  # heads per core
QKV_COLS = 3 * HPC * D  # 1536
SCALE = 1.0 / math.sqrt(D)
ROPE_BASE = 10000.0

_BUILD_CACHE = {}


def build(S, phases=('qkv', 'attn', 'proj'), reps=1):
    TOK = B * S
    TS = TOK // N_CORES  # per-core token slice == QKV token-block width
    assert TS <= 512 and S % TS == 0
    NTB = N_CORES
    KT = HID // 128  # 32 contraction tiles
    NKT8 = S // 128  # k-token tiles per attention instance
    NQH = max(1, S // 512)  # q halves per attention instance
    QW = min(S, 512)
    f32, f32r, bf16 = dt.float32, dt.float32r, dt.bfloat16
    RG = [list(range(N_CORES))]

    nc = bacc.Bacc("TRN2", target_bir_lowering=False, debug=False,
                   num_devices=N_CORES)

    # ---- I/O
    cos_in = nc.dram_tensor("cos_t", [RH, TOK], f32, kind="ExternalInput")
    sin_in = nc.dram_tensor("sin_t", [RH, TOK], f32, kind="ExternalInput")
    hidT_in = nc.dram_tensor("hidT", [HID, TOK], f32r, kind="ExternalInput")
    wqkv_in = nc.dram_tensor("w_qkv_sh", [HID, QKV_COLS], f32r, kind="ExternalInput")
    wout_in = nc.dram_tensor("w_out_full", [HID, HID], bf16, kind="ExternalInput")
    out_f = nc.dram_tensor("out_f", [TS, HID], f32, kind="ExternalOutput")

    # ---- internal DRAM
    qkvT_d = nc.dram_tensor("qkvT_d", [2 * HPC * D, TOK], f32r)
    vtok_d = [nc.dram_tensor(f"vtok_d{h}", [TOK, D], f32r) for h in range(HPC)]
    a2a_in = [nc.dram_tensor(f"a2a_in{h}", [N_CORES, D, TS], bf16) for h in range(HPC)]
    a2a_out = [nc.dram_tensor(f"a2a_out{h}", [N_CORES, D, TS], bf16)
               for h in range(HPC)]

    with tile.TileContext(nc) as tc:
        ctx_pool = tc.tile_pool(name="const", bufs=1)
        with ctx_pool as cpool:
            ident = cpool.tile([128, 128], f32)
            make_identity(nc, ident[:])
            ident_r = cpool.tile([128, 128], f32r)
            nc.vector.tensor_copy(out=ident_r[:], in_=ident[:])
            ones_f = cpool.tile([128, 1], f32)
            nc.vector.memset(ones_f[:], 1.0)
            ones_r = cpool.tile([128, 1], f32r)
            nc.vector.tensor_copy(out=ones_r[:], in_=ones_f[:])
            tri_f = cpool.tile([128, 128], f32)
            make_upper_triangular(nc, tri_f[:], val=1.0, diag=True)
            tri_r = cpool.tile([128, 128], f32r)
            nc.vector.tensor_copy(out=tri_r[:], in_=tri_f[:])
            for rep in range(reps):
                # ---- phase 1: QKV projection (transposed), two column halves
                NM = 6
                KB = 8  # k-tiles per rhs block: same-psum matmul runs
                qkv_passes = range(2) if 'qkv' in phases else range(0)
                ROPE_GC = {g * 256 for g in range(2 * HPC)}  # first 128 cols of q/k blocks
                pf_cm = tc.tile_pool(name=f"qkv_wpf_{rep}", bufs=1) \
                    if 'qkv' in phases else None
                pf_pool = pf_cm.__enter__() if pf_cm is not None else None
                pf_tiles = []
                for p in qkv_passes:
                    with tc.tile_pool(name=f"qkv_w{p}_{rep}", bufs=1) as wq_pool, \
                         tc.tile_pool(name=f"qkv_rhs{p}_{rep}", bufs=2) as rhs_pool, \
                         tc.tile_pool(name=f"qkv_st{p}_{rep}", bufs=2) as stg_pool, \
                         tc.tile_pool(name=f"qkv_ps{p}_{rep}", bufs=1, space="PSUM") as qps_pool:
                        # weight tiles for this pass; kb-block 0 either comes
                        # from the prefetch issued during the previous pass or
                        # is loaded first, before the bulk of the weights.
                        w_sb = list(pf_tiles)
                        pf_tiles = []
                        kt_lo = len(w_sb)

                        def load_w(kt, pool=wq_pool, p=p):
                            w = pool.tile([128, NM * 128], f32r, tag=f"w{p}_{kt}",
                                          name=f"w{p}_{kt}_r{rep}")
                            nc.sync.dma_start(
                                out=w[:],
                                in_=wqkv_in.ap()[128 * kt:128 * (kt + 1),
                                                 NM * 128 * p:NM * 128 * (p + 1)])
                            return w

                        for kt in range(kt_lo, KB):
                            w_sb.append(load_w(kt))
                        for tb in range(NTB):
                            ps = [qps_pool.tile([128, TS], dt.float32, tag=f"qkvps{m}",
                                                name=f"qkvps{m}_{p}_{tb}_r{rep}")
                                  for m in range(NM)]
                            for kb in range(KT // KB):
                                kts_blk = list(range(KB * kb, KB * (kb + 1)))
                                blk = rhs_pool.tile([128, KB * TS], f32r, tag="qkvrhs",
                                                    name=f"qkvrhs_{p}_{tb}_{kb}_r{rep}")
                                for i, kt in enumerate(kts_blk):
                                    nc.sync.dma_start(
                                        out=blk[:, TS * i:TS * (i + 1)],
                                        in_=hidT_in.ap()[128 * kt:128 * (kt + 1),
                                                         TS * tb:TS * (tb + 1)])
                                if tb == 0 and kb == 0 and len(w_sb) < KT:
                                    # bulk weight load, behind the first rhs block
                                    for kt in range(KB, KT):
                                        w_sb.append(load_w(kt))
                                for m in range(NM):
                                    for i, kt in enumerate(kts_blk):
                                        nc.tensor.matmul(
                                            out=ps[m][:],
                                            lhsT=w_sb[kt][:, 128 * m:128 * (m + 1)],
                                            rhs=blk[:, TS * i:TS * (i + 1)],
                                            start=(kb == 0 and i == 0),
                                            stop=(kb == KT // KB - 1 and i == KB - 1))
                            if p == 0 and tb == NTB - 2:
                                # prefetch next pass's first weight block while
                                # the last two token blocks of pass 0 compute
                                pf_tiles = [load_w(kt, pool=pf_pool, p=1)
                                            for kt in range(KB)]
                            for m in range(NM):
                                gc = NM * 128 * p + 128 * m
                                dst = stg_pool.tile([128, TS], f32r, tag=f"qst{m}")
                                if gc in ROPE_GC:
                                    cs_ = stg_pool.tile([RH, TS], f32, tag="cs",
                                                        name=f"cs_{p}_{tb}_{m}_r{rep}")
                                    sn_ = stg_pool.tile([RH, TS], f32, tag="sn",
                                                        name=f"sn_{p}_{tb}_{m}_r{rep}")
                                    nc.sync.dma_start(out=cs_[:],
                                                      in_=cos_in.ap()[:, TS * tb:TS * (tb + 1)])
                                    nc.sync.dma_start(out=sn_[:],
                                                      in_=sin_in.ap()[:, TS * tb:TS * (tb + 1)])
                                    c = cs_[:]
                                    s = sn_[:]
                                    t1 = stg_pool.tile([RH, TS], f32, tag="rt1")
                                    t2 = stg_pool.tile([RH, TS], f32, tag="rt2")
                                    nc.vector.tensor_mul(t1[:], ps[m][0:RH, :], c)
                                    nc.vector.tensor_mul(t2[:], ps[m][RH:2 * RH, :], s)
                                    nc.vector.tensor_sub(dst[0:RH, :], t1[:], t2[:])
                                    t3 = stg_pool.tile([RH, TS], f32, tag="rt3")
                                    t4 = stg_pool.tile([RH, TS], f32, tag="rt4")
                                    nc.vector.tensor_mul(t3[:], ps[m][RH:2 * RH, :], c)
                                    nc.vector.tensor_mul(t4[:], ps[m][0:RH, :], s)
                                    nc.vector.tensor_add(dst[RH:2 * RH, :], t3[:], t4[:])
                                else:
                                    nc.vector.tensor_copy(out=dst[:], in_=ps[m][:])
                                if gc < 2 * HPC * D:
                                    nc.sync.dma_start(
                                        out=qkvT_d.ap()[gc:gc + 128, TS * tb:TS * (tb + 1)],
                                        in_=dst[:])
                                else:
                                    hl_ = (gc - 2 * HPC * D) // D
                                    d0 = (gc - 2 * HPC * D) % D
                                    for q8 in range((TS + 127) // 128):
                                        qq = min(128, TS - 128 * q8)
                                        tpv = qps_pool.tile([128, 128], f32r, tag="vtp",
                                                            name=f"vtp_{p}_{tb}_{m}_{q8}_r{rep}",
                                                            bufs=2)
                                        nc.tensor.transpose(
                                            tpv[0:qq, :],
                                            dst[:, 128 * q8:128 * q8 + qq],
                                            ident_r[:])
                                        vst = stg_pool.tile([128, 128], f32r, tag="vst")
                                        nc.vector.tensor_copy(out=vst[0:qq, :],
                                                              in_=tpv[0:qq, :])
                                        tok0 = TS * tb + 128 * q8
                                        nc.sync.dma_start(
                                            out=vtok_d[hl_].ap()[tok0:tok0 + qq, d0:d0 + 128],
                                            in_=vst[0:qq, :])
                if pf_cm is not None:
                    pf_cm.__exit__(None, None, None)

                # ---- phase 2: attention per (local head, batch)
                attn_on = 'attn' in phases
                with tc.tile_pool(name=f"att_in_{rep}", bufs=2) as ain_pool, \
                     tc.tile_pool(name=f"att_vt_{rep}", bufs=2) as avt_pool, \
                     tc.tile_pool(name=f"att_pr_{rep}", bufs=2) as apr_pool, \
                     tc.tile_pool(name=f"att_o_{rep}", bufs=2) as aout_pool, \
                     tc.tile_pool(name=f"att_sc_{rep}", bufs=2, space="PSUM") as scps_pool, \
                     tc.tile_pool(name=f"att_av_{rep}", bufs=1, space="PSUM") as avps_pool:
                    for hl in (range(HPC) if attn_on else range(0)):
                        for b in range(B):
                            qoff = D * hl
                            koff = HPC * D + D * hl
                            tok0 = S * b

                            def load_pair(off, nm):
                                ts_ = []
                                for dtile in range(2):
                                    t = ain_pool.tile([128, S], f32r, tag=f"{nm}{dtile}")
                                    nc.sync.dma_start(
                                        out=t[:],
                                        in_=qkvT_d.ap()[off + 128 * dtile:off + 128 * (dtile + 1),
                                                        tok0:tok0 + S])
                                    ts_.append(t)
                                return ts_

                            qT = load_pair(qoff, "q")
                            kT = load_pair(koff, "k")

                            # token-major v tiles (transposed during QKV phase)
                            vtok = []
                            for kt8 in range(NKT8):
                                vt = avt_pool.tile([128, D], f32r, tag=f"vtok{kt8}")
                                nc.sync.dma_start(
                                    out=vt[:],
                                    in_=vtok_d[hl].ap()[tok0 + 128 * kt8:tok0 + 128 * (kt8 + 1), :])
                                vtok.append(vt)

                            # scoresT -> exp -> probsT
                            probsT = []
                            for kt8 in range(NKT8):
                                pr = apr_pool.tile([128, S], f32r, tag=f"pr{kt8}")
                                qlo = 128 * kt8
                                q0 = qlo
                                while q0 < S:
                                    wch = min(512, S - q0)
                                    pss = scps_pool.tile([128, QW], dt.float32, tag="scps")
                                    for dtile in range(2):
                                        nc.tensor.matmul(
                                            out=pss[:, 0:wch],
                                            lhsT=kT[dtile][:, 128 * kt8:128 * (kt8 + 1)],
                                            rhs=qT[dtile][:, q0:q0 + wch],
                                            start=(dtile == 0), stop=(dtile == 1))
                                    nc.scalar.activation(
                                        out=pr[:, q0:q0 + wch], in_=pss[:, 0:wch],
                                        func=mybir.ActivationFunctionType.Exp, scale=SCALE)
                                    q0 += wch
                                nc.vector.tensor_mul(pr[:, qlo:qlo + 128],
                                                     pr[:, qlo:qlo + 128], tri_r[:])
                                probsT.append(pr)

                            # PV + denominator
                            ps_av = [[avps_pool.tile([128, QW], dt.float32, tag=f"av{d}{q}",
                                                      name=f"av{d}{q}_{hl}_{b}_r{rep}")
                                      for q in range(NQH)] for d in range(2)]
                            ps_sum = [avps_pool.tile([1, QW], dt.float32, tag=f"sm{q}",
                                                     name=f"sm{q}_{hl}_{b}_r{rep}")
                                      for q in range(NQH)]
                            pv_work = {}
                            for qh in range(NQH):
                                q0, q1 = QW * qh, QW * (qh + 1)
                                last_kt = min(NKT8 - 1, (q1 - 1) // 128)
                                pv_work[qh] = [
                                    (kt8, q0, q1, max(128 * kt8, q0),
                                     kt8 == 0, kt8 == last_kt)
                                    for kt8 in range(NKT8)
                                    if max(128 * kt8, q0) < q1]
                            for dtile in range(2):
                                for qh in range(NQH):
                                    for kt8, q0, q1, lo, st, sp in pv_work[qh]:
                                        nc.tensor.matmul(
                                            out=ps_av[dtile][qh][:, lo - q0:q1 - q0],
                                            lhsT=vtok[kt8][:, 128 * dtile:128 * (dtile + 1)],
                                            rhs=probsT[kt8][:, lo:q1], start=st, stop=sp)
                            for qh in range(NQH):
                                for kt8, q0, q1, lo, st, sp in pv_work[qh]:
                                    nc.tensor.matmul(out=ps_sum[qh][:, lo - q0:q1 - q0],
                                                     lhsT=ones_r[:],
                                                     rhs=probsT[kt8][:, lo:q1],
                                                     start=st, stop=sp)

                            # evict PV and denominators out of PSUM promptly
                            # (scalar engine) so the next (head, batch) can
                            # reuse the banks without waiting on normalize
                            av_sb = [[aout_pool.tile([128, QW], f32, tag=f"avs{d}{q}",
                                                     name=f"avs{d}{q}_{hl}_{b}_r{rep}")
                                      for q in range(NQH)] for d in range(2)]
                            sums_sb = aout_pool.tile([1, S], f32, tag="sums")
                            for qh in range(NQH):
                                nc.scalar.copy(out=sums_sb[:, QW * qh:QW * (qh + 1)],
                                               in_=ps_sum[qh][:])
                            for dtile in range(2):
                                for qh in range(NQH):
                                    nc.scalar.copy(out=av_sb[dtile][qh][:],
                                                   in_=ps_av[dtile][qh][:])
                            recip = aout_pool.tile([1, S], f32, tag="recip")
                            nc.vector.reciprocal(out=recip[:], in_=sums_sb[:])
                            rbc = aout_pool.tile([128, S], f32, tag="rbc")
                            nc.gpsimd.partition_broadcast(rbc[:], recip[0:1, :])
                            for dtile in range(2):
                                att_sb = aout_pool.tile([128, S], bf16, tag=f"attn{dtile}")
                                for qh in range(NQH):
                                    q0, q1 = QW * qh, QW * (qh + 1)
                                    nc.vector.tensor_mul(att_sb[:, q0:q1],
                                                         av_sb[dtile][qh][:],
                                                         rbc[:, q0:q1])
                                for u in range(S // TS):
                                    dest = (S * b) // TS + u
                                    nc.sync.dma_start(
                                        out=a2a_in[hl].ap()[dest,
                                                            128 * dtile:128 * (dtile + 1), :],
                                        in_=att_sb[:, TS * u:TS * (u + 1)])
                        nc.gpsimd.collective_compute(
                            "AllToAll", mybir.AluOpType.bypass, replica_groups=RG,
                            ins=[a2a_in[hl].ap().opt()], outs=[a2a_out[hl].ap().opt()])

                # ---- phase 3: output projection for own token slice,
                # hl=0 pass (accumulate to SBUF) then hl=1 pass (add on top)
                NMT = (TS + 127) // 128
                NNT = HID // 512 if 'proj' in phases else 0
                PKB = 4  # k-tiles per w block
                with tc.tile_pool(name=f"op_a_{rep}", bufs=1) as oa_pool, \
                     tc.tile_pool(name=f"op_w_{rep}", bufs=6) as ow_pool, \
                     tc.tile_pool(name=f"op_acc_{rep}", bufs=1) as acc_pool, \
                     tc.tile_pool(name=f"op_f_{rep}", bufs=3) as of_pool, \
                     tc.tile_pool(name=f"op_ps_{rep}", bufs=2, space="PSUM") as ops_pool:
                    acc = {}
                    for hl in (range(HPC) if 'proj' in phases else range(0)):
                        kts = [(hl, src, sub) for src in range(N_CORES)
                               for sub in range(2)]
                        NKB = (len(kts) + PKB - 1) // PKB

                        def load_wblk(nt, kb, hl=hl, kts=kts):
                            kis = list(range(PKB * kb, min(PKB * (kb + 1), len(kts))))
                            wblk = ow_pool.tile([128, PKB * 512], bf16, tag="wr",
                                                name=f"wr_{hl}_{nt}_{kb}_r{rep}")
                            for i, ki in enumerate(kis):
                                _, src, sub = kts[ki]
                                r0 = 512 * src + 256 * hl + 128 * sub
                                nc.sync.dma_start(
                                    out=wblk[:, 512 * i:512 * (i + 1)],
                                    in_=wout_in.ap()[r0:r0 + 128,
                                                     512 * nt:512 * (nt + 1)])
                            return kis, wblk

                        # nt=0 weight blocks first: they don't depend on the
                        # collective, so they must go into the DMA queues
                        # ahead of the a2a-gated attn tile loads below.
                        wblks0 = [load_wblk(0, kb) for kb in range(NKB)]
                        am = {}
                        for src in range(N_CORES):
                            for sub in range(2):
                                t = oa_pool.tile([128, TS], bf16,
                                                 tag=f"am{hl}_{src}_{sub}")
                                nc.sync.dma_start(
                                    out=t[:],
                                    in_=a2a_out[hl].ap()[src, 128 * sub:128 * (sub + 1), :])
                                am[(hl, src, sub)] = t
                        for nt in range(NNT):
                            ps_f = [ops_pool.tile([min(128, TS), 512], dt.float32,
                                                  tag=f"f{mt}",
                                                  name=f"f{hl}_{mt}_{nt}_r{rep}")
                                    for mt in range(NMT)]
                            for kb in range(NKB):
                                kis, wblk = wblks0[kb] if nt == 0 else load_wblk(nt, kb)
                                for mt in range(NMT):
                                    mm = min(128, TS - 128 * mt)
                                    for i, ki in enumerate(kis):
                                        nc.tensor.matmul(
                                            out=ps_f[mt][:],
                                            lhsT=am[kts[ki]][:, 128 * mt:128 * mt + mm],
                                            rhs=wblk[:, 512 * i:512 * (i + 1)],
                                            start=(ki == 0), stop=(ki == len(kts) - 1))
                            for mt in range(NMT):
                                mm = min(128, TS - 128 * mt)
                                if hl == 0:
                                    a = acc_pool.tile([min(128, TS), 512], f32,
                                                      tag=f"acc{mt}_{nt}")
                                    nc.scalar.copy(out=a[:], in_=ps_f[mt][:])
                                    acc[(mt, nt)] = a
                                else:
                                    fo = of_pool.tile([min(128, TS), 512], f32, tag="fo")
                                    nc.vector.tensor_add(fo[:], ps_f[mt][:],
                                                         acc[(mt, nt)][:])
                                    nc.sync.dma_start(
                                        out=out_f.ap()[128 * mt:128 * mt + mm,
                                                       512 * nt:512 * (nt + 1)],
                                        in_=fo[:])

    nc.compile()
    return nc


def get_nc(S):
    if S not in _BUILD_CACHE:
        _BUILD_CACHE[S] = build(S)
    return _BUILD_CACHE[S]


def make_in_maps(position_ids, hidden_states, w_qkv, w_out):
    S = hidden_states.shape[1]
    TOK = B * S
    flat = np.asarray(hidden_states, dtype=np.float32).reshape(TOK, HID)
    hidT = np.ascontiguousarray(flat.T)
    pos = np.asarray(position_ids).reshape(TOK).astype(np.float32)
    invf = (1.0 / (ROPE_BASE ** (np.arange(0, ROT, 2, dtype=np.float32) / ROT)))
    ang = invf[:, None] * pos[None, :]  # [RH, TOK]
    cos_t = np.cos(ang).astype(np.float32)
    sin_t = np.sin(ang).astype(np.float32)
    w_qkv = np.asarray(w_qkv, dtype=np.float32)
    bf16_np = mybir.dt.np(dt.bfloat16)
    w_out = np.ascontiguousarray(np.asarray(w_out, dtype=np.float32).astype(bf16_np))
    in_maps = []
    for c in range(N_CORES):
        c0 = HPC * D * c
        wq = np.concatenate([w_qkv[:, c0:c0 + HPC * D],
                             w_qkv[:, HID + c0:HID + c0 + HPC * D],
                             w_qkv[:, 2 * HID + c0:2 * HID + c0 + HPC * D]], axis=1)
        in_maps.append({
            "cos_t": cos_t,
            "sin_t": sin_t,
            "hidT": hidT,
            "w_qkv_sh": np.ascontiguousarray(wq),
            "w_out_full": w_out,
        })
    return in_maps


def kernel(position_ids, hidden_states, w_qkv, w_out):
    S = hidden_states.shape[1]
    nc = get_nc(S)
    in_maps = make_in_maps(position_ids, hidden_states, w_qkv, w_out)
    res = run_bass_kernel_spmd(nc, in_maps, list(range(N_CORES)))
    TOK = B * S
    out = np.concatenate([res.results[c]["out_f"] for c in range(N_CORES)], axis=0)
    return out.reshape(B, S, HID).astype(np.float32)
